# revision 17
# baseline (speedup 1.0000x reference)
"""Trainium2 Bass kernel for the recurrent-SE / depthwise-conv attention block.

Math per layer (faithful to the reference):
    pooled = mean(x, (2,3))                      # [B, C]
    ht, ct = cell(pooled, ht, ct)                # DSU cell, state [B, C]
    out_h, _ = cell(pooled, ht[0], ct[0])        # GLOBAL batch-0 state bcast
    x = x * (1 + out_h)[:, :, None, None] + dwconv3x3(x)

Sharding: data-parallel over batch, 8 samples/core. Because the second cell
reads the global sample-0 recurrent state, every core also carries a replica
of sample 0 (9 sample-plane-sets per core); the replica planes evolve like
any other sample so ht[0]/ct[0] stay locally available.

Per core:
  - x lives in SBUF in a zero-padded [30x30] per-(channel-block, sample)
    plane layout, channels on partitions (4 blocks of 128 channels).
  - dwconv3x3 runs on the TensorEngine as 9 accumulating matmuls per
    half-plane chunk with host-prebuilt diagonal tap matrices (fp32r).
  - The combine x*(1+out_h) + conv is one fused DVE scalar_tensor_tensor per
    half-plane whose accum_out also yields the pooled sums for the next
    layer (the 1/784 mean is folded into w_ih_l1 host-side).
  - The DSU cell runs on tiny fp32 matmuls (pre-transposed weights) plus
    ACT sigmoid/tanh.
"""

import numpy as np

import concourse.bacc as bacc
import concourse.bass as bass
import concourse.mybir as mybir
import concourse.tile as tile
from concourse.bass_utils import run_bass_kernel_spmd

F32 = mybir.dt.float32
F32R = mybir.dt.float32r
ALU = mybir.AluOpType
ACTF = mybir.ActivationFunctionType
AX = mybir.AxisListType

N_CORES = 8
B_FULL, C, H, W = 64, 512, 28, 28
B_SH = B_FULL // N_CORES           # 8 shard samples per core
BL = B_SH + 1                      # +1 replica of global sample 0
CB = C // 128                      # 4 channel blocks
HW = H * W                         # 784
PR, PC = H + 2, W + 2              # padded plane 30 x 30
PLANE = PR * PC                    # 900
NPLANE = CB * BL                   # 36 planes per core
HALF = H // 2                      # 14 rows per half-plane chunk
NCHUNK = HALF * W                  # 392 columns per conv matmul


def build_program(num_layers: int = 4, iters: int = 1):
    nc = bacc.Bacc("TRN2", target_bir_lowering=False, debug=False,
                   num_devices=N_CORES)

    x_d = nc.dram_tensor("x", [BL, C, H, W], F32, kind="ExternalInput").ap()
    diag_d = nc.dram_tensor("diag", [CB * 9 * 128, 128], F32R,
                            kind="ExternalInput").ap()
    wih1t_d = nc.dram_tensor("wih1t", [C, 32], F32, kind="ExternalInput").ap()
    whh1t_d = nc.dram_tensor("whh1t", [C, 32], F32, kind="ExternalInput").ap()
    wih2t_d = nc.dram_tensor("wih2t", [32, 3 * C], F32, kind="ExternalInput").ap()
    whh2t_d = nc.dram_tensor("whh2t", [32, 3 * C], F32, kind="ExternalInput").ap()
    bsum_d = nc.dram_tensor("bsum", [12, 128], F32, kind="ExternalInput").ap()
    b1_d = nc.dram_tensor("b1", [32, 2], F32, kind="ExternalInput").ap()
    y_d = nc.dram_tensor("y", [B_SH, C, H, W], F32, kind="ExternalOutput").ap()

    with tile.TileContext(nc) as tc:
        with (
            tc.tile_pool(name="persist", bufs=1) as pp,
            tc.tile_pool(name="stagep", bufs=2) as sp,
            tc.tile_pool(name="convps", bufs=5, space="PSUM") as cvp,
            tc.tile_pool(name="cellps", bufs=1, space="PSUM") as clp,
        ):
            # +PC slack so the last plane's shifted window slice stays in range
            xpad = pp.tile([128, NPLANE * PLANE + PC], F32R, tag="xpad")
            diag_sb = pp.tile([128, CB * 9 * 128], F32R, tag="diag")
            wih1t_sb = pp.tile([128, CB * 32], F32, tag="wih1t")
            whh1t_sb = pp.tile([128, CB * 32], F32, tag="whh1t")
            wih2t_sb = pp.tile([32, 3 * C], F32, tag="wih2t")
            whh2t_sb = pp.tile([32, 3 * C], F32, tag="whh2t")
            bsum_sb = pp.tile([128, 12], F32, tag="bsum")
            b1_sb = pp.tile([32, 2], F32, tag="b1")

            pooled = pp.tile([128, NPLANE], F32, tag="pooled")
            poolacc = pp.tile([128, NPLANE * 2], F32, tag="poolacc")
            ht = pp.tile([128, NPLANE], F32, tag="ht")
            ct = pp.tile([128, NPLANE], F32, tag="ct")
            z1 = pp.tile([32, 2 * BL + 1], F32, tag="z1")
            gates = pp.tile([128, 3 * NPLANE], F32, tag="gates")
            sgi = pp.tile([128, 3 * NPLANE], F32, tag="sgi")
            tmt = pp.tile([128, 3 * NPLANE], F32, tag="tmt")
            tmp2 = pp.tile([128, 12], F32, tag="tmp2")
            gi_sb = pp.tile([128, 3 * NPLANE], F32, tag="gi_sb")
            s_sb = pp.tile([128, NPLANE], F32, tag="s_sb")

            z1ps = clp.tile([32, 2 * BL + 1], F32, tag="z1ps")
            gips = clp.tile([128, 3 * NPLANE], F32, tag="gips")
            ghps = clp.tile([128, 3 * NPLANE + 12], F32, tag="ghps")

            # constants in once
            nc.sync.dma_start(
                diag_sb[:, :].rearrange("p (blk m) -> p blk m", m=128),
                diag_d.rearrange("(blk k) m -> k blk m", k=128))
            nc.sync.dma_start(
                wih1t_sb[:, :].rearrange("p (cb m) -> p cb m", m=32),
                wih1t_d.rearrange("(cb k) m -> k cb m", k=128))
            nc.sync.dma_start(
                whh1t_sb[:, :].rearrange("p (cb m) -> p cb m", m=32),
                whh1t_d.rearrange("(cb k) m -> k cb m", k=128))
            nc.sync.dma_start(wih2t_sb[:, :], wih2t_d)
            nc.sync.dma_start(whh2t_sb[:, :], whh2t_d)
            nc.sync.dma_start(bsum_sb[:, :], bsum_d.rearrange("j p -> p j"))
            nc.sync.dma_start(b1_sb[:, :], b1_d)
            # zero the padded buffer once; interiors are overwritten each
            # layer, borders stay zero forever. (memset can't write f32r, so
            # zero a small f32 tile and fan it out through DVE copies)
            z0 = pp.tile([128, PLANE + PC], F32, tag="z0")
            nc.vector.memset(z0[:, :], 0.0)
            for pl in range(NPLANE):
                nc.vector.tensor_copy(
                    xpad[:, pl * PLANE:(pl + 1) * PLANE], z0[:, 0:PLANE])
            nc.vector.tensor_copy(
                xpad[:, NPLANE * PLANE:NPLANE * PLANE + PC], z0[:, 0:PC])

            flat = xpad[:, :]

            def intr(pl, r0, nr):
                """interior window [128, nr, 28] of plane pl at row r0."""
                off = pl * PLANE + (r0 + 1) * PC + 1
                return flat[:, off:off + nr * PC].rearrange(
                    "p (r w) -> p r w", w=PC)[:, :, 0:W]

            def shifted(pl, r0, dy, dx):
                off = pl * PLANE + (r0 + 1 + dy) * PC + 1 + dx
                return flat[:, off:off + HALF * PC].rearrange(
                    "p (r w) -> p r w", w=PC)[:, :, 0:W]

            def emit_cell(first_layer: bool):
                """pooled, ht, ct -> new ht, ct; s = 1 + out_h."""
                for cb in range(CB):
                    nc.tensor.matmul(
                        z1ps[:, 0:BL], wih1t_sb[:, cb * 32:(cb + 1) * 32],
                        pooled[:, cb * BL:(cb + 1) * BL],
                        start=(cb == 0), stop=(cb == CB - 1))
                if first_layer:
                    # ht == 0 -> hh path of cell 1 contributes relu(b_hh1)
                    nc.vector.memset(z1ps[:, BL:2 * BL], 0.0)
                else:
                    for cb in range(CB):
                        nc.tensor.matmul(
                            z1ps[:, BL:2 * BL],
                            whh1t_sb[:, cb * 32:(cb + 1) * 32],
                            ht[:, cb * BL:(cb + 1) * BL],
                            start=(cb == 0), stop=(cb == CB - 1))
                nc.scalar.activation(z1[:, 0:BL], z1ps[:, 0:BL], ACTF.Relu,
                                     bias=b1_sb[:, 0:1])
                nc.scalar.activation(z1[:, BL:2 * BL], z1ps[:, BL:2 * BL],
                                     ACTF.Relu, bias=b1_sb[:, 1:2])
                for g in range(3):
                    for cb in range(CB):
                        co = (g * CB + cb) * BL
                        wsl = slice(g * C + cb * 128, g * C + (cb + 1) * 128)
                        nc.tensor.matmul(
                            gips[:, co:co + BL], wih2t_sb[:, wsl],
                            z1[:, 0:BL], start=True, stop=True)
                        nc.tensor.matmul(
                            ghps[:, co:co + BL], whh2t_sb[:, wsl],
                            z1[:, BL:2 * BL], start=True, stop=True)
                # evict g_i to SBUF (reused by cell 2; also avoids the
                # two-PSUM-source restriction on scalar_tensor_tensor)
                nc.vector.tensor_copy(gi_sb[:, :], gips[:, :])
                # gates1 = g_i + g_h + (b_ih_l2 + b_hh_l2)
                for g in range(3):
                    for cb in range(CB):
                        co = (g * CB + cb) * BL
                        nc.vector.scalar_tensor_tensor(
                            gates[:, co:co + BL], gi_sb[:, co:co + BL],
                            bsum_sb[:, g * CB + cb:g * CB + cb + 1],
                            ghps[:, co:co + BL], ALU.add, ALU.add)
                G = NPLANE  # 36 cols per gate
                nc.scalar.activation(sgi[:, 0:G], gates[:, 0:G], ACTF.Sigmoid)
                nc.scalar.activation(sgi[:, G:2 * G], gates[:, G:2 * G],
                                     ACTF.Sigmoid)
                nc.scalar.activation(sgi[:, 2 * G:3 * G], gates[:, 2 * G:3 * G],
                                     ACTF.Tanh)
                # ncx = sig(f)*ct + sig(i)*tanh(c);  new ct = ncx
                nc.vector.tensor_tensor(tmt[:, 0:G], sgi[:, 0:G],
                                        sgi[:, 2 * G:3 * G], ALU.mult)
                if first_layer:
                    nc.vector.tensor_copy(ct[:, :], tmt[:, 0:G])
                else:
                    nc.vector.tensor_tensor(tmt[:, G:2 * G], sgi[:, G:2 * G],
                                            ct[:, :], ALU.mult)
                    nc.vector.tensor_tensor(ct[:, :], tmt[:, 0:G],
                                            tmt[:, G:2 * G], ALU.add)
                nc.scalar.activation(ht[:, :], ct[:, :], ACTF.Sigmoid)

                # ---- second cell: hx = ht[0], cx = ct[0] (broadcast) ----
                for cb in range(CB):
                    nc.tensor.matmul(
                        z1ps[:, 2 * BL:2 * BL + 1],
                        whh1t_sb[:, cb * 32:(cb + 1) * 32],
                        ht[:, cb * BL:cb * BL + 1],
                        start=(cb == 0), stop=(cb == CB - 1))
                nc.scalar.activation(z1[:, 2 * BL:2 * BL + 1],
                                     z1ps[:, 2 * BL:2 * BL + 1], ACTF.Relu,
                                     bias=b1_sb[:, 1:2])
                for g in range(3):
                    for cb in range(CB):
                        j = 3 * NPLANE + g * CB + cb
                        nc.tensor.matmul(
                            ghps[:, j:j + 1],
                            whh2t_sb[:, g * C + cb * 128:g * C + (cb + 1) * 128],
                            z1[:, 2 * BL:2 * BL + 1], start=True, stop=True)
                nc.vector.tensor_tensor(tmp2[:, :],
                                        ghps[:, 3 * NPLANE:3 * NPLANE + 12],
                                        bsum_sb[:, :], ALU.add)
                # gates2 = g_i + (g_h2 + bias), g_h2 broadcast over batch
                for g in range(3):
                    for cb in range(CB):
                        co = (g * CB + cb) * BL
                        nc.vector.tensor_scalar(
                            gates[:, co:co + BL], gi_sb[:, co:co + BL],
                            tmp2[:, g * CB + cb:g * CB + cb + 1], None,
                            ALU.add)
                nc.scalar.activation(sgi[:, 0:G], gates[:, 0:G], ACTF.Sigmoid)
                nc.scalar.activation(sgi[:, G:2 * G], gates[:, G:2 * G],
                                     ACTF.Sigmoid)
                nc.scalar.activation(sgi[:, 2 * G:3 * G], gates[:, 2 * G:3 * G],
                                     ACTF.Tanh)
                nc.vector.tensor_tensor(tmt[:, 0:G], sgi[:, 0:G],
                                        sgi[:, 2 * G:3 * G], ALU.mult)
                # ncx2 = sig(f2)*ct[0] + sig(i2)*tanh(c2)
                for cb in range(CB):
                    bs = cb * BL
                    nc.vector.scalar_tensor_tensor(
                        tmt[:, G + bs:G + bs + BL], sgi[:, G + bs:G + bs + BL],
                        ct[:, bs:bs + 1], tmt[:, bs:bs + BL],
                        ALU.mult, ALU.add)
                nc.scalar.activation(tmt[:, 2 * G:3 * G], tmt[:, G:2 * G],
                                     ACTF.Sigmoid)
                nc.vector.tensor_scalar(s_sb[:, :], tmt[:, 2 * G:3 * G], 1.0,
                                        None, ALU.add)

            def emit_body():
                # ---- load x: DRAM -> stage -> padded layout; pooled(1) ----
                for cb in range(CB):
                    for b in range(BL):
                        pl = cb * BL + b
                        stage = sp.tile([128, HW], F32, tag="stage")
                        nc.sync.dma_start(
                            stage[:, :],
                            x_d[b, cb * 128:(cb + 1) * 128, :, :].rearrange(
                                "c h w -> c (h w)"))
                        nc.vector.tensor_reduce(
                            pooled[:, pl:pl + 1], stage[:, :], AX.X, ALU.add)
                        # out marked f32r: values feeding fp32r matmuls must
                        # be rounded to the PE's reduced fp32 format
                        nc.scalar.copy(
                            intr(pl, 0, H),
                            stage[:, :].rearrange("p (h w) -> p h w", w=W))

                for layer in range(num_layers):
                    last = layer == num_layers - 1
                    if layer > 0:
                        pa = poolacc[:, :].rearrange("p (n h) -> p n h", h=2)
                        for cb in range(CB):
                            nc.vector.tensor_tensor(
                                pooled[:, cb * BL:(cb + 1) * BL],
                                pa[:, cb * BL:(cb + 1) * BL, 0],
                                pa[:, cb * BL:(cb + 1) * BL, 1],
                                ALU.add)
                    emit_cell(first_layer=(layer == 0))

                    for cb in range(CB):
                        for b in range(BL):
                            if last and b == 0:
                                continue  # replica plane: no output needed
                            pl = cb * BL + b
                            # both halves' convs must run before either
                            # combine overwrites the shared halo rows 13/14
                            pss = []
                            for hf in range(2):
                                r0 = hf * HALF
                                ps = cvp.tile([128, NCHUNK], F32, tag="cps")
                                pss.append(ps)
                                for t in range(9):
                                    dy, dx = t // 3 - 1, t % 3 - 1
                                    nc.tensor.matmul(
                                        ps[:, :],
                                        diag_sb[:, (cb * 9 + t) * 128:
                                                (cb * 9 + t + 1) * 128],
                                        shifted(pl, r0, dy, dx),
                                        start=(t == 0), stop=(t == 8))
                            # final layer writes full-precision results to a
                            # contiguous staging tile (xpad stays f32r-only),
                            # other layers update xpad in place, rounded to
                            # f32r for the next layer's conv.
                            ost = (sp.tile([128, HW], F32, tag="stage",
                                           name="ost")
                                   if last else None)
                            for hf in range(2):
                                r0 = hf * HALF
                                ps = pss[hf]
                                src = intr(pl, r0, HALF)
                                if last:
                                    dst = ost[:, r0 * W:(r0 + HALF) * W
                                              ].rearrange(
                                        "p (r w) -> p r w", w=W)
                                    acc = None
                                else:
                                    dst = src
                                    acc = poolacc[:, pl * 2 + hf:
                                                  pl * 2 + hf + 1]
                                nc.vector.scalar_tensor_tensor(
                                    dst, src, s_sb[:, pl:pl + 1],
                                    ps[:, :].rearrange("p (r w) -> p r w", w=W),
                                    ALU.mult, ALU.add, accum_out=acc)
                            if last:
                                nc.sync.dma_start(
                                    y_d[b - 1, cb * 128:(cb + 1) * 128, :, :],
                                    ost[:, :].rearrange(
                                        "p (h w) -> p h w", w=W))

            if iters == 1:
                emit_body()
            else:
                with tc.For_i(0, iters, 1):
                    emit_body()

    nc.compile()
    return nc


def prep_inputs(x, w_ih_l1, b_ih_l1, w_ih_l2, b_ih_l2,
                w_hh_l1, b_hh_l1, w_hh_l2, b_hh_l2, dw_kernel):
    """Host-side prep: per-core input maps (weights replicated)."""
    x = np.ascontiguousarray(np.asarray(x, dtype=np.float32))
    diag = np.zeros((CB, 9, 128, 128), np.float32)
    dw = np.asarray(dw_kernel, np.float32).reshape(C, 9)
    idx = np.arange(128)
    for cb in range(CB):
        for t in range(9):
            diag[cb, t, idx, idx] = dw[cb * 128:(cb + 1) * 128, t]
    common = {
        "diag": diag.reshape(CB * 9 * 128, 128),
        "wih1t": np.ascontiguousarray(
            (np.asarray(w_ih_l1, np.float32) / HW).T),
        "whh1t": np.ascontiguousarray(np.asarray(w_hh_l1, np.float32).T),
        "wih2t": np.ascontiguousarray(np.asarray(w_ih_l2, np.float32).T),
        "whh2t": np.ascontiguousarray(np.asarray(w_hh_l2, np.float32).T),
        "bsum": np.ascontiguousarray(
            (np.asarray(b_ih_l2, np.float32)
             + np.asarray(b_hh_l2, np.float32)).reshape(3, CB, 128)
            .reshape(12, 128)),
        "b1": np.ascontiguousarray(np.stack(
            [np.asarray(b_ih_l1, np.float32),
             np.asarray(b_hh_l1, np.float32)], axis=1)),
    }
    return [dict(common, x=np.ascontiguousarray(np.concatenate(
        [x[0:1], x[i * B_SH:(i + 1) * B_SH]], axis=0)))
        for i in range(N_CORES)]


_cache = {}


def kernel(**inputs) -> np.ndarray:
    num_layers = int(inputs["num_layers"])
    if num_layers not in _cache:
        _cache[num_layers] = build_program(num_layers=num_layers, iters=1)
    nc = _cache[num_layers]
    in_maps = prep_inputs(
        inputs["x"], inputs["w_ih_l1"], inputs["b_ih_l1"], inputs["w_ih_l2"],
        inputs["b_ih_l2"], inputs["w_hh_l1"], inputs["b_hh_l1"],
        inputs["w_hh_l2"], inputs["b_hh_l2"], inputs["dw_kernel"])
    res = run_bass_kernel_spmd(nc, in_maps, list(range(N_CORES)))
    return np.concatenate([res.results[i]["y"] for i in range(N_CORES)],
                          axis=0).astype(np.float32)


# revision 22
# speedup vs baseline: 1.1411x; 1.1411x over previous
"""Trainium2 Bass kernel for the recurrent-SE / depthwise-conv attention block.

Math per layer (faithful to the reference):
    pooled = mean(x, (2,3))                      # [B, C]
    ht, ct = cell(pooled, ht, ct)                # DSU cell, state [B, C]
    out_h, _ = cell(pooled, ht[0], ct[0])        # GLOBAL batch-0 state bcast
    x = x * (1 + out_h)[:, :, None, None] + dwconv3x3(x)

Sharding: data-parallel over batch, 8 samples/core. Because the second cell
reads the global sample-0 recurrent state, every core also carries a replica
of sample 0 (9 sample-plane-sets per core); the replica planes evolve like
any other sample so ht[0]/ct[0] stay locally available.

Per core:
  - x lives in SBUF in a zero-padded [30x30] per-(channel-block, sample)
    plane layout, channels on partitions (4 blocks of 128 channels), stored
    as f32r (the PE's reduced fp32) since it feeds fp32r conv matmuls.
  - dwconv3x3 runs on the TensorEngine as 9 accumulating matmuls per
    half-plane chunk with host-prebuilt diagonal tap matrices (fp32r).
  - The combine x*(1+out_h) + conv is one fused DVE scalar_tensor_tensor per
    half-plane whose accum_out also yields the pooled sums for the next
    layer (the 1/784 mean is folded into w_ih_l1 host-side).
  - The DSU cell runs on tiny fp32 matmuls; the l2 gate biases ride in an
    augmented ones-row of z1 so the gate sums need no separate bias ops.
    Cell matmul groups are interleaved between conv planes so the PE FIFO
    never idles on the serial cell chain.
"""

import numpy as np

import concourse.bacc as bacc
import concourse.bass as bass
import concourse.mybir as mybir
import concourse.tile as tile
from concourse.bass_utils import run_bass_kernel_spmd

F32 = mybir.dt.float32
F32R = mybir.dt.float32r
ALU = mybir.AluOpType
ACTF = mybir.ActivationFunctionType
AX = mybir.AxisListType

N_CORES = 8
B_FULL, C, H, W = 64, 512, 28, 28
B_SH = B_FULL // N_CORES           # 8 shard samples per core
BL = B_SH + 1                      # +1 replica of global sample 0
CB = C // 128                      # 4 channel blocks
HW = H * W                         # 784
PR, PC = H + 2, W + 2              # padded plane 30 x 30
PLANE = PR * PC                    # 900
NPLANE = CB * BL                   # 36 planes per core
HALF = H // 2                      # 14 rows per half-plane chunk
NCHUNK = HALF * W                  # 392 columns per conv matmul
G3 = 3 * NPLANE                    # 108 gate columns

# packed single-bank cell PSUM layout (columns of cellps)
ZC1 = 2 * BL + 1                   # z1 pre-activations [33p, 19]
GI0, GI1 = ZC1, ZC1 + G3           # g_i (+bias) 12 x 9
GH0, GH1 = GI1, GI1 + G3           # g_h 12 x 9
G20, G21 = GH1, GH1 + 12           # g_h2 (batch-0 bcast) 12 x 1


def build_program(num_layers: int = 4, iters: int = 1,
                  skip_cells: bool = False, skip_io: bool = False):
    nc = bacc.Bacc("TRN2", target_bir_lowering=False, debug=False,
                   num_devices=N_CORES)

    x_d = nc.dram_tensor("x", [BL, C, H, W], F32, kind="ExternalInput").ap()
    diag_d = nc.dram_tensor("diag", [CB * 9 * 128, 128], F32R,
                            kind="ExternalInput").ap()
    wih1t_d = nc.dram_tensor("wih1t", [C, 32], F32, kind="ExternalInput").ap()
    whh1t_d = nc.dram_tensor("whh1t", [C, 32], F32, kind="ExternalInput").ap()
    wih2t_d = nc.dram_tensor("wih2t", [33, 3 * C], F32, kind="ExternalInput").ap()
    whh2t_d = nc.dram_tensor("whh2t", [33, 3 * C], F32, kind="ExternalInput").ap()
    b1_d = nc.dram_tensor("b1", [32, 2], F32, kind="ExternalInput").ap()
    y_d = nc.dram_tensor("y", [B_SH, C, H, W], F32, kind="ExternalOutput").ap()

    with tile.TileContext(nc) as tc:
        with (
            tc.tile_pool(name="persist", bufs=1) as pp,
            tc.tile_pool(name="stagep", bufs=3) as sp,
            tc.tile_pool(name="convps", bufs=7, space="PSUM") as cvp,
            tc.tile_pool(name="cellps", bufs=1, space="PSUM") as clp,
        ):
            # +PC slack so the last plane's shifted window slice stays in range
            xpad = pp.tile([128, NPLANE * PLANE + PC], F32R, tag="xpad")
            diag_sb = pp.tile([128, CB * 9 * 128], F32R, tag="diag")
            wih1t_sb = pp.tile([128, CB * 32], F32, tag="wih1t")
            whh1t_sb = pp.tile([128, CB * 32], F32, tag="whh1t")
            wih2t_sb = pp.tile([33, 3 * C], F32, tag="wih2t")
            whh2t_sb = pp.tile([33, 3 * C], F32, tag="whh2t")
            b1_sb = pp.tile([32, 2], F32, tag="b1")

            pooled = pp.tile([128, NPLANE], F32, tag="pooled")
            poolacc = pp.tile([128, NPLANE * 2], F32, tag="poolacc")
            ht = pp.tile([128, NPLANE], F32, tag="ht")
            ct = pp.tile([128, NPLANE], F32, tag="ct")
            z1 = pp.tile([33, 2 * BL + 1], F32, tag="z1")
            gates = pp.tile([128, G3], F32, tag="gates")
            sgi = pp.tile([128, G3], F32, tag="sgi")
            tmt = pp.tile([128, G3], F32, tag="tmt")
            gi_sb = pp.tile([128, G3], F32, tag="gi_sb")
            s_sb = pp.tile([128, NPLANE], F32, tag="s_sb")

            cellps = clp.tile([128, G21], F32, tag="cellps")

            # constants in once
            nc.sync.dma_start(
                diag_sb[:, :].rearrange("p (blk m) -> p blk m", m=128),
                diag_d.rearrange("(blk k) m -> k blk m", k=128))
            nc.sync.dma_start(
                wih1t_sb[:, :].rearrange("p (cb m) -> p cb m", m=32),
                wih1t_d.rearrange("(cb k) m -> k cb m", k=128))
            nc.sync.dma_start(
                whh1t_sb[:, :].rearrange("p (cb m) -> p cb m", m=32),
                whh1t_d.rearrange("(cb k) m -> k cb m", k=128))
            nc.sync.dma_start(wih2t_sb[:, :], wih2t_d)
            nc.sync.dma_start(whh2t_sb[:, :], whh2t_d)
            nc.sync.dma_start(b1_sb[:, :], b1_d)
            # ones row for the augmented-bias matmuls
            nc.vector.memset(z1[32:33, :], 1.0)
            # zero the padded buffer once; interiors are overwritten each
            # layer, borders stay zero forever. (memset can't write f32r, so
            # zero a small f32 tile and fan it out through DVE copies)
            z0 = sp.tile([128, PLANE + PC], F32, tag="stage", name="z0")
            nc.vector.memset(z0[:, :], 0.0)
            for pl in range(NPLANE):
                nc.vector.tensor_copy(
                    xpad[:, pl * PLANE:(pl + 1) * PLANE], z0[:, 0:PLANE])
            nc.vector.tensor_copy(
                xpad[:, NPLANE * PLANE:NPLANE * PLANE + PC], z0[:, 0:PC])

            flat = xpad[:, :]

            def intr(pl, r0, nr):
                """interior window [128, nr, 28] of plane pl at row r0."""
                off = pl * PLANE + (r0 + 1) * PC + 1
                return flat[:, off:off + nr * PC].rearrange(
                    "p (r w) -> p r w", w=PC)[:, :, 0:W]

            def shifted(pl, r0, dy, dx):
                off = pl * PLANE + (r0 + 1 + dy) * PC + 1 + dx
                return flat[:, off:off + HALF * PC].rearrange(
                    "p (r w) -> p r w", w=PC)[:, :, 0:W]

            def cell_stage(st, first_layer):
                """The DSU cell in 5 stages so PE work can interleave with
                conv planes.  pooled, ht, ct -> new ht, ct; s = 1+out_h."""
                if st == 0:  # z1 pre-activations (PE)
                    for cb in range(CB):
                        nc.tensor.matmul(
                            cellps[0:32, 0:BL],
                            wih1t_sb[:, cb * 32:(cb + 1) * 32],
                            pooled[:, cb * BL:(cb + 1) * BL],
                            start=(cb == 0), stop=(cb == CB - 1))
                    if first_layer:
                        # ht == 0 -> hh path contributes relu(b_hh1)
                        nc.vector.memset(cellps[0:32, BL:2 * BL], 0.0)
                    else:
                        for cb in range(CB):
                            nc.tensor.matmul(
                                cellps[0:32, BL:2 * BL],
                                whh1t_sb[:, cb * 32:(cb + 1) * 32],
                                ht[:, cb * BL:(cb + 1) * BL],
                                start=(cb == 0), stop=(cb == CB - 1))
                elif st == 1:  # relu, then gate matmuls (PE bulk)
                    nc.scalar.activation(z1[0:32, 0:BL], cellps[0:32, 0:BL],
                                         ACTF.Relu, bias=b1_sb[:, 0:1])
                    nc.scalar.activation(z1[0:32, BL:2 * BL],
                                         cellps[0:32, BL:2 * BL],
                                         ACTF.Relu, bias=b1_sb[:, 1:2])
                    for g in range(3):
                        for cb in range(CB):
                            co = (g * CB + cb) * BL
                            wsl = slice(g * C + cb * 128,
                                        g * C + (cb + 1) * 128)
                            nc.tensor.matmul(
                                cellps[:, GI0 + co:GI0 + co + BL],
                                wih2t_sb[:, wsl], z1[:, 0:BL],
                                start=True, stop=True)
                            nc.tensor.matmul(
                                cellps[:, GH0 + co:GH0 + co + BL],
                                whh2t_sb[:, wsl], z1[:, BL:2 * BL],
                                start=True, stop=True)
                elif st == 2:  # cell 1 state update (DVE/ACT)
                    nc.vector.tensor_copy(gi_sb[:, :], cellps[:, GI0:GI1])
                    nc.vector.tensor_tensor(gates[:, :], gi_sb[:, :],
                                            cellps[:, GH0:GH1], ALU.add)
                    nc.scalar.activation(sgi[:, 0:NPLANE], gates[:, 0:NPLANE],
                                         ACTF.Sigmoid)
                    nc.scalar.activation(sgi[:, NPLANE:2 * NPLANE],
                                         gates[:, NPLANE:2 * NPLANE],
                                         ACTF.Sigmoid)
                    nc.scalar.activation(sgi[:, 2 * NPLANE:G3],
                                         gates[:, 2 * NPLANE:G3], ACTF.Tanh)
                    nc.vector.tensor_tensor(tmt[:, 0:NPLANE], sgi[:, 0:NPLANE],
                                            sgi[:, 2 * NPLANE:G3], ALU.mult)
                    if first_layer:
                        nc.vector.tensor_copy(ct[:, :], tmt[:, 0:NPLANE])
                    else:
                        nc.vector.tensor_tensor(
                            tmt[:, NPLANE:2 * NPLANE],
                            sgi[:, NPLANE:2 * NPLANE], ct[:, :], ALU.mult)
                        nc.vector.tensor_tensor(ct[:, :], tmt[:, 0:NPLANE],
                                                tmt[:, NPLANE:2 * NPLANE],
                                                ALU.add)
                    nc.scalar.activation(ht[:, :], ct[:, :], ACTF.Sigmoid)
                elif st == 3:  # cell 2 hh path from ht[0] (PE)
                    for cb in range(CB):
                        nc.tensor.matmul(
                            cellps[0:32, 2 * BL:2 * BL + 1],
                            whh1t_sb[:, cb * 32:(cb + 1) * 32],
                            ht[:, cb * BL:cb * BL + 1],
                            start=(cb == 0), stop=(cb == CB - 1))
                    nc.scalar.activation(z1[0:32, 2 * BL:2 * BL + 1],
                                         cellps[0:32, 2 * BL:2 * BL + 1],
                                         ACTF.Relu, bias=b1_sb[:, 1:2])
                    for g in range(3):
                        for cb in range(CB):
                            j = G20 + g * CB + cb
                            nc.tensor.matmul(
                                cellps[:, j:j + 1],
                                whh2t_sb[:, g * C + cb * 128:
                                         g * C + (cb + 1) * 128],
                                z1[:, 2 * BL:2 * BL + 1],
                                start=True, stop=True)
                else:  # st == 4: cell 2 -> s = 1 + out_h (DVE/ACT)
                    # gates2 = (g_i + bias) + g_h2 broadcast over batch
                    nc.vector.tensor_tensor(
                        gates[:, :].rearrange("p (j b) -> p j b", b=BL),
                        gi_sb[:, :].rearrange("p (j b) -> p j b", b=BL),
                        cellps[:, G20:G21].unsqueeze(-1).broadcast_to(
                            [128, 12, BL]),
                        ALU.add)
                    nc.scalar.activation(sgi[:, 0:NPLANE], gates[:, 0:NPLANE],
                                         ACTF.Sigmoid)
                    nc.scalar.activation(sgi[:, NPLANE:2 * NPLANE],
                                         gates[:, NPLANE:2 * NPLANE],
                                         ACTF.Sigmoid)
                    nc.scalar.activation(sgi[:, 2 * NPLANE:G3],
                                         gates[:, 2 * NPLANE:G3], ACTF.Tanh)
                    nc.vector.tensor_tensor(tmt[:, 0:NPLANE], sgi[:, 0:NPLANE],
                                            sgi[:, 2 * NPLANE:G3], ALU.mult)
                    # ncx2 = sig(f2)*ct[0] + sig(i2)*tanh(c2)
                    for cb in range(CB):
                        bs = cb * BL
                        nc.vector.scalar_tensor_tensor(
                            tmt[:, NPLANE + bs:NPLANE + bs + BL],
                            sgi[:, NPLANE + bs:NPLANE + bs + BL],
                            ct[:, bs:bs + 1], tmt[:, bs:bs + BL],
                            ALU.mult, ALU.add)
                    nc.scalar.activation(tmt[:, 2 * NPLANE:G3],
                                         tmt[:, NPLANE:2 * NPLANE],
                                         ACTF.Sigmoid)
                    nc.vector.tensor_scalar(s_sb[:, :], tmt[:, 2 * NPLANE:G3],
                                            1.0, None, ALU.add)

            def emit_conv(pl):
                """the 18 conv matmuls of one plane -> 2 psum tiles."""
                cb = pl // BL
                pss = []
                for hf in range(2):
                    r0 = hf * HALF
                    ps = cvp.tile([128, NCHUNK], F32, tag="cps", name="cps")
                    pss.append(ps)
                    for t in range(9):
                        dy, dx = t // 3 - 1, t % 3 - 1
                        nc.tensor.matmul(
                            ps[:, :],
                            diag_sb[:, (cb * 9 + t) * 128:
                                    (cb * 9 + t + 1) * 128],
                            shifted(pl, r0, dy, dx),
                            start=(t == 0), stop=(t == 8))
                return pss

            def emit_combine(pl, pss, last):
                cb = pl // BL
                ost = (sp.tile([128, HW], F32, tag="stage", name="ost")
                       if last else None)
                for hf in range(2):
                    r0 = hf * HALF
                    ps = pss[hf]
                    src = intr(pl, r0, HALF)
                    if last:
                        dst = ost[:, r0 * W:(r0 + HALF) * W].rearrange(
                            "p (r w) -> p r w", w=W)
                        acc = None
                    else:
                        dst = src
                        acc = poolacc[:, pl * 2 + hf:pl * 2 + hf + 1]
                    nc.vector.scalar_tensor_tensor(
                        dst, src, s_sb[:, pl:pl + 1],
                        ps[:, :].rearrange("p (r w) -> p r w", w=W),
                        ALU.mult, ALU.add, accum_out=acc)
                if last:
                    b = pl % BL
                    nc.sync.dma_start(
                        y_d[b - 1, cb * 128:(cb + 1) * 128, :, :],
                        ost[:, :].rearrange("p (h w) -> p h w", w=W))

            def emit_plane(pl, last):
                emit_combine(pl, emit_conv(pl), last)

            def emit_input():
                # DRAM -> stage (paired planes, both HWDGE rings) -> padded
                # layout; also the layer-1 pooled sums
                for cb in range(CB):
                    groups = [(0, 2), (2, 2), (4, 2), (6, 2), (8, 1)]
                    for gi, (b0, nb) in enumerate(groups):
                        stage = sp.tile([128, 2 * HW], F32, tag="stage",
                                        name="stage")
                        eng = nc.sync if gi % 2 == 0 else nc.scalar
                        eng.dma_start(
                            stage[:, 0:nb * HW].rearrange(
                                "p (b hw) -> p b hw", hw=HW),
                            x_d[b0:b0 + nb, cb * 128:(cb + 1) * 128, :, :]
                            .rearrange("b c h w -> c b (h w)"))
                        for k in range(nb):
                            pl = cb * BL + b0 + k
                            seg = stage[:, k * HW:(k + 1) * HW]
                            nc.vector.tensor_reduce(
                                pooled[:, pl:pl + 1], seg, AX.X, ALU.add)
                            nc.scalar.copy(
                                intr(pl, 0, H),
                                seg.rearrange("p (h w) -> p h w", w=W))

            if skip_io:
                emit_input()
            if skip_cells:
                nc.vector.memset(s_sb[:, :], 1.5)

            def emit_body():
                if not skip_io:
                    emit_input()
                for layer in range(num_layers):
                    last = layer == num_layers - 1
                    if layer > 0 and not skip_cells:
                        # pooled = half0 + half1 of the combine accumulators
                        nc.vector.tensor_tensor(
                            pooled[:, :],
                            poolacc[:, 0:2 * NPLANE:2],
                            poolacc[:, 1:2 * NPLANE:2],
                            ALU.add)
                    planes = [cb * BL + b for cb in range(CB)
                              for b in range(BL)
                              if not (last and b == 0)]
                    if skip_cells:
                        for pl in planes:
                            emit_plane(pl, last)
                        continue
                    # interleave the serial cell chain with the first conv
                    # planes so the PE FIFO doesn't idle on it; their
                    # combines wait until s is available (emitted after
                    # stage 4 so the dependency binds to THIS layer's s)
                    early = planes[0:3]
                    held = []
                    for st in range(5):
                        cell_stage(st, layer == 0)
                        if st < len(early):
                            held.append((early[st], emit_conv(early[st])))
                    for pl, pss in held:
                        emit_combine(pl, pss, last)
                    for pl in planes[len(early):]:
                        emit_plane(pl, last)

            if iters == 1:
                emit_body()
            else:
                with tc.For_i(0, iters, 1):
                    emit_body()

    nc.compile()
    return nc


def prep_inputs(x, w_ih_l1, b_ih_l1, w_ih_l2, b_ih_l2,
                w_hh_l1, b_hh_l1, w_hh_l2, b_hh_l2, dw_kernel):
    """Host-side prep: per-core input maps (weights replicated)."""
    x = np.ascontiguousarray(np.asarray(x, dtype=np.float32))
    diag = np.zeros((CB, 9, 128, 128), np.float32)
    dw = np.asarray(dw_kernel, np.float32).reshape(C, 9)
    idx = np.arange(128)
    for cb in range(CB):
        for t in range(9):
            diag[cb, t, idx, idx] = dw[cb * 128:(cb + 1) * 128, t]
    # l2 weights pre-transposed with the summed gate bias as an extra row
    # (pairs with the ones-row of z1)
    wih2t = np.concatenate(
        [np.asarray(w_ih_l2, np.float32).T,
         (np.asarray(b_ih_l2, np.float32)
          + np.asarray(b_hh_l2, np.float32))[None, :]], axis=0)
    whh2t = np.concatenate(
        [np.asarray(w_hh_l2, np.float32).T,
         np.zeros((1, 3 * C), np.float32)], axis=0)
    common = {
        "diag": diag.reshape(CB * 9 * 128, 128),
        "wih1t": np.ascontiguousarray(
            (np.asarray(w_ih_l1, np.float32) / HW).T),
        "whh1t": np.ascontiguousarray(np.asarray(w_hh_l1, np.float32).T),
        "wih2t": np.ascontiguousarray(wih2t),
        "whh2t": np.ascontiguousarray(whh2t),
        "b1": np.ascontiguousarray(np.stack(
            [np.asarray(b_ih_l1, np.float32),
             np.asarray(b_hh_l1, np.float32)], axis=1)),
    }
    return [dict(common, x=np.ascontiguousarray(np.concatenate(
        [x[0:1], x[i * B_SH:(i + 1) * B_SH]], axis=0)))
        for i in range(N_CORES)]


_cache = {}


def kernel(**inputs) -> np.ndarray:
    num_layers = int(inputs["num_layers"])
    if num_layers not in _cache:
        _cache[num_layers] = build_program(num_layers=num_layers, iters=1)
    nc = _cache[num_layers]
    in_maps = prep_inputs(
        inputs["x"], inputs["w_ih_l1"], inputs["b_ih_l1"], inputs["w_ih_l2"],
        inputs["b_ih_l2"], inputs["w_hh_l1"], inputs["b_hh_l1"],
        inputs["w_hh_l2"], inputs["b_hh_l2"], inputs["dw_kernel"])
    res = run_bass_kernel_spmd(nc, in_maps, list(range(N_CORES)))
    return np.concatenate([res.results[i]["y"] for i in range(N_CORES)],
                          axis=0).astype(np.float32)
